# revision 11
# baseline (speedup 1.0000x reference)
"""GRU (B=256, T=2048, H=100) Trainium2 kernel.

Data-parallel over batch: 8 NeuronCores x 32 batch rows each. The
sequential scan over T=2048 steps runs device-local on each core.

Layout: hidden state kept transposed [H=100 partitions, B=32 free].
Per step, three matmuls with augmented stationary weights
lhsT_g = [W_hh_g.T ; b_g ; W_ih_g.T]  (K=103, M=100) against
rhs = [h_{t-1} ; 1 ; v_t] [103, 32] produce r_pre | z_pre | hn in one
PSUM tile; sigmoid/tanh on ScalarE, gate arithmetic on VectorE. Each
step's h_t is written into the next step's rhs block of a persistent
SBUF region, which doubles as the hs buffer for the output projection
(out = w_out @ h + b_out, bias folded via the region's ones row).
"""

import sys

sys.path.insert(0, "/opt/trn_rl_repo")

import numpy as np

B, T, H = 256, 2048, 100
NCORES = 8
BL = B // NCORES          # 32 batch rows per core
TC = 128                  # steps per chunk
NCHUNK = T // TC
AUG = H + 3               # h rows + ones row + 2 v rows
ONES_ROW = H              # partition index of the ones row
V_ROW = H + 1             # partition index of first v row

_compiled = None


def _build_kernel():
    import concourse.mybir as mybir
    from concourse import bacc
    from concourse.tile import TileContext

    fp32 = mybir.dt.float32
    nc = bacc.Bacc(None, target_bir_lowering=False)

    # DRAM I/O (per-core tensors; same names across cores for SPMD)
    d_wr = nc.dram_tensor("wr", [AUG, H], fp32, kind="ExternalInput")
    d_wz = nc.dram_tensor("wz", [AUG, H], fp32, kind="ExternalInput")
    d_wn = nc.dram_tensor("wn", [AUG, H], fp32, kind="ExternalInput")
    d_xw = nc.dram_tensor("xw", [2, H], fp32, kind="ExternalInput")
    d_xb = nc.dram_tensor("xb", [H, 1], fp32, kind="ExternalInput")
    d_ow = nc.dram_tensor("ow", [H + 1, 2], fp32, kind="ExternalInput")
    d_blk0 = nc.dram_tensor("blk0", [AUG, BL], fp32, kind="ExternalInput")
    d_vT = nc.dram_tensor("vT", [3, T * BL], fp32, kind="ExternalInput")
    d_ones = nc.dram_tensor("ones_tail", [1, BL], fp32, kind="ExternalInput")
    d_out = nc.dram_tensor("outT", [2, T * BL], fp32, kind="ExternalOutput")

    RW = (TC + 1) * BL    # region width
    XW = TC * BL          # xn buffer width

    with TileContext(nc) as tc:
        with (
            tc.tile_pool(name="const", bufs=1) as cpool,
            tc.tile_pool(name="reg", bufs=1) as rpool,
            tc.tile_pool(name="xn", bufs=1) as xpool,
            tc.tile_pool(name="gates", bufs=4) as gpool,
            tc.tile_pool(name="psg", bufs=4, space="PSUM") as pgpool,
            tc.tile_pool(name="psx", bufs=2, space="PSUM") as pxpool,
            tc.tile_pool(name="pso", bufs=2, space="PSUM") as popool,
            tc.tile_pool(name="outs", bufs=4) as opool,
        ):
            # --- constants into SBUF ---
            wr = cpool.tile([AUG, H], fp32, tag="wr")
            wz = cpool.tile([AUG, H], fp32, tag="wz")
            wn = cpool.tile([AUG, H], fp32, tag="wn")
            xw = cpool.tile([2, H], fp32, tag="xw")
            xb = cpool.tile([H, 1], fp32, tag="xb")
            ow = cpool.tile([H + 1, 2], fp32, tag="ow")
            nc.sync.dma_start(out=wr, in_=d_wr[:, :])
            nc.sync.dma_start(out=wz, in_=d_wz[:, :])
            nc.sync.dma_start(out=wn, in_=d_wn[:, :])
            nc.sync.dma_start(out=xw, in_=d_xw[:, :])
            nc.sync.dma_start(out=xb, in_=d_xb[:, :])
            nc.sync.dma_start(out=ow, in_=d_ow[:, :])

            # --- persistent ping-pong buffers ---
            regs = [
                rpool.tile([AUG, RW], fp32, name="regA", tag="regA"),
                rpool.tile([AUG, RW], fp32, name="regB", tag="regB"),
            ]
            vxs = [
                xpool.tile([2, XW], fp32, name="vxA", tag="vxA"),
                xpool.tile([2, XW], fp32, name="vxB", tag="vxB"),
            ]
            xns = [
                xpool.tile([H, XW], fp32, name="xnA", tag="xnA"),
                xpool.tile([H, XW], fp32, name="xnB", tag="xnB"),
            ]
            # ones row of the trailing block (written once per region)
            nc.sync.dma_start(
                out=regs[0][ONES_ROW : ONES_ROW + 1, XW:RW], in_=d_ones[:, :]
            )
            nc.sync.dma_start(
                out=regs[1][ONES_ROW : ONES_ROW + 1, XW:RW], in_=d_ones[:, :]
            )
            # chunk 0 block 0: pre-assembled [h0; 1; v_0]
            nc.sync.dma_start(out=regs[0][0:AUG, 0:BL], in_=d_blk0[:, :])

            def load_chunk_v(k):
                """DMA chunk k's [ones; v] rows into the region + v copy for xn."""
                c0 = 0 if k > 0 else BL  # chunk 0 block 0 comes from blk0
                nc.sync.dma_start(
                    out=regs[k % 2][ONES_ROW:AUG, c0:XW],
                    in_=d_vT[:, k * XW + c0 : (k + 1) * XW],
                )
                nc.sync.dma_start(
                    out=vxs[k % 2][:, :], in_=d_vT[1:3, k * XW : (k + 1) * XW]
                )

            def compute_xn(k):
                """xn = W_in @ v + b_in for chunk k (K=2 matmuls), into SBUF."""
                vx = vxs[k % 2]
                xnb = xns[k % 2]
                for j in range(0, XW, 512):
                    w = min(512, XW - j)
                    ps = pxpool.tile([H, 512], fp32, tag="psx")
                    nc.tensor.matmul(
                        ps[:, :w], xw, vx[:, j : j + w], start=True, stop=True
                    )
                    # PSUM -> SBUF with the b_in bias folded in
                    nc.scalar.add(out=xnb[:, j : j + w], in_=ps[:, :w], add=xb[:, 0:1])

            load_chunk_v(0)
            compute_xn(0)

            for k in range(NCHUNK):
                reg = regs[k % 2]
                xnb = xns[k % 2]

                # prefetch next chunk's v + xn while this chunk scans
                if k + 1 < NCHUNK:
                    load_chunk_v(k + 1)
                    compute_xn(k + 1)

                for t in range(TC):
                    c0 = t * BL
                    c1 = c0 + BL
                    rhs = reg[0:AUG, c0:c1]
                    pg = pgpool.tile([H, 3 * BL], fp32, name="pg", tag="pg")
                    nc.tensor.matmul(pg[:, 0:BL], wr, rhs, start=True, stop=True)
                    nc.tensor.matmul(
                        pg[:, BL : 2 * BL], wz, rhs, start=True, stop=True
                    )
                    nc.tensor.matmul(
                        pg[:, 2 * BL : 3 * BL], wn, rhs, start=True, stop=True
                    )
                    sig = gpool.tile([H, 2 * BL], fp32, name="sig", tag="sig")
                    nc.scalar.activation(
                        sig, pg[:, 0 : 2 * BL], mybir.ActivationFunctionType.Sigmoid
                    )
                    m1 = gpool.tile([H, BL], fp32, name="m1", tag="m1")
                    nc.vector.tensor_mul(m1, sig[:, 0:BL], pg[:, 2 * BL : 3 * BL])
                    npre = gpool.tile([H, BL], fp32, name="npre", tag="npre")
                    nc.vector.tensor_add(npre, m1, xnb[:, c0:c1])
                    nt = gpool.tile([H, BL], fp32, name="nt", tag="nt")
                    nc.scalar.activation(
                        nt, npre, mybir.ActivationFunctionType.Tanh
                    )
                    s1 = gpool.tile([H, BL], fp32, name="s1", tag="s1")
                    nc.vector.tensor_sub(s1, reg[0:H, c0:c1], nt)  # h - n
                    m2 = gpool.tile([H, BL], fp32, name="m2", tag="m2")
                    nc.vector.tensor_mul(m2, sig[:, BL : 2 * BL], s1)  # z*(h-n)
                    # h' = n + z*(h-n) -> next rhs block
                    nc.vector.tensor_add(reg[0:H, c1 : c1 + BL], nt, m2)

                # carry h into next chunk's block 0
                if k + 1 < NCHUNK:
                    nc.vector.tensor_copy(
                        regs[(k + 1) % 2][0:H, 0:BL], reg[0:H, TC * BL : RW]
                    )

                # output projection for this chunk: blocks 1..TC hold
                # h_t for global steps k*TC .. k*TC+TC-1
                for j in range(0, XW, 512):
                    w = min(512, XW - j)
                    po = popool.tile([2, 512], fp32, tag="po")
                    nc.tensor.matmul(
                        po[:, :w],
                        ow,
                        reg[0 : H + 1, BL + j : BL + j + w],
                        start=True,
                        stop=True,
                    )
                    ob = opool.tile([2, 512], fp32, tag="ob")
                    nc.vector.tensor_copy(ob[:, :w], po[:, :w])
                    nc.sync.dma_start(
                        out=d_out[:, k * XW + j : k * XW + j + w], in_=ob[:, :w]
                    )

    return nc


def _prep_inputs(x_i, v, w_ih, w_hh, b_ih, b_hh, w_out, b_out):
    """Host-side prep: h0 least-squares init + per-core shards."""
    f = np.float32
    x_i, v = np.asarray(x_i, f), np.asarray(v, f)
    w_ih, w_hh = np.asarray(w_ih, f), np.asarray(w_hh, f)
    b_ih, b_hh = np.asarray(b_ih, f), np.asarray(b_hh, f)
    w_out, b_out = np.asarray(w_out, f), np.asarray(b_out, f)

    A = w_out @ w_out.T
    bb = np.linalg.solve(A.astype(np.float64), (x_i - b_out).T.astype(np.float64))
    h0 = (bb.T @ w_out.astype(np.float64)).astype(f)  # [B, H]

    def aug(g0, with_ih):
        whh = w_hh[g0 : g0 + H]                      # [H, H]
        if with_ih:
            b = b_ih[g0 : g0 + H] + b_hh[g0 : g0 + H]
            wih = w_ih[g0 : g0 + H]                  # [H, 2]
        else:
            b = b_hh[g0 : g0 + H]
            wih = np.zeros((H, 2), f)
        out = np.empty((AUG, H), f)
        out[0:H] = whh.T
        out[ONES_ROW] = b
        out[V_ROW : V_ROW + 2] = wih.T
        return np.ascontiguousarray(out)

    wr = aug(0, True)
    wz = aug(H, True)
    wn = aug(2 * H, False)
    xw = np.ascontiguousarray(w_ih[2 * H : 3 * H].T)          # [2, H]
    xb = np.ascontiguousarray(b_ih[2 * H : 3 * H].reshape(H, 1))
    ow = np.empty((H + 1, 2), f)
    ow[0:H] = w_out.T
    ow[H] = b_out
    ow = np.ascontiguousarray(ow)
    ones_tail = np.ones((1, BL), f)

    in_maps = []
    for c in range(NCORES):
        rows = slice(c * BL, (c + 1) * BL)
        vT = np.empty((3, T * BL), f)
        vT[0] = 1.0
        vT[1:3] = v[rows].transpose(2, 1, 0).reshape(2, T * BL)
        vT = np.ascontiguousarray(vT)
        blk0 = np.empty((AUG, BL), f)
        blk0[0:H] = h0[rows].T
        blk0[ONES_ROW] = 1.0
        blk0[V_ROW : V_ROW + 2] = vT[1:3, 0:BL]
        blk0 = np.ascontiguousarray(blk0)
        in_maps.append(
            {
                "wr": wr, "wz": wz, "wn": wn, "xw": xw, "xb": xb, "ow": ow,
                "blk0": blk0, "vT": vT, "ones_tail": ones_tail,
            }
        )
    return in_maps


def kernel(x_i, v, w_ih, w_hh, b_ih, b_hh, w_out, b_out, trace=False, tmpdir=None):
    global _compiled
    from concourse.bass_utils import run_bass_kernel_spmd

    in_maps = _prep_inputs(x_i, v, w_ih, w_hh, b_ih, b_hh, w_out, b_out)
    if _compiled is None:
        _compiled = _build_kernel()
        _compiled.finalize()
    kw = {}
    if trace:
        kw = dict(trace=True, tmpdir=tmpdir)
    res = run_bass_kernel_spmd(
        _compiled, in_maps, core_ids=list(range(NCORES)), **kw
    )
    out = np.empty((B, T, 2), np.float32)
    for c in range(NCORES):
        outT = res.results[c]["outT"]                              # [2, T*BL]
        out[c * BL : (c + 1) * BL] = outT.reshape(2, T, BL).transpose(2, 1, 0)
    kernel.last_results = res
    return out


# revision 12
# speedup vs baseline: 1.0797x; 1.0797x over previous
"""GRU (B=256, T=2048, H=100) Trainium2 kernel.

Data-parallel over batch: 8 NeuronCores x 32 batch rows each. The
sequential scan over T=2048 steps runs device-local on each core.

Layout: hidden state kept transposed [H=100 partitions, B=32 free].
Per step, three matmuls with augmented stationary weights
lhsT_g = [W_hh_g.T ; b_g ; W_ih_g.T]  (K=103, M=100) against
rhs = [h_{t-1} ; 1 ; v_t] [103, 32] produce r_pre | z_pre | hn in one
PSUM tile; sigmoid/tanh on ScalarE, gate arithmetic on VectorE. Each
step's h_t is written into the next step's rhs block of a persistent
SBUF region, which doubles as the hs buffer for the output projection
(out = w_out @ h + b_out, bias folded via the region's ones row).
"""

import sys

sys.path.insert(0, "/opt/trn_rl_repo")

import numpy as np

B, T, H = 256, 2048, 100
NCORES = 8
BL = B // NCORES          # 32 batch rows per core
TC = 128                  # steps per chunk
NCHUNK = T // TC
AUG = H + 3               # h rows + ones row + 2 v rows
ONES_ROW = H              # partition index of the ones row
V_ROW = H + 1             # partition index of first v row

_compiled = None


def _build_kernel():
    import concourse.mybir as mybir
    from concourse import bacc
    from concourse.tile import TileContext

    fp32 = mybir.dt.float32
    nc = bacc.Bacc(None, target_bir_lowering=False)

    # DRAM I/O (per-core tensors; same names across cores for SPMD)
    d_wr = nc.dram_tensor("wr", [AUG, H], fp32, kind="ExternalInput")
    d_wz = nc.dram_tensor("wz", [AUG, H], fp32, kind="ExternalInput")
    d_wn = nc.dram_tensor("wn", [AUG, H], fp32, kind="ExternalInput")
    d_xw = nc.dram_tensor("xw", [2, H], fp32, kind="ExternalInput")
    d_xb = nc.dram_tensor("xb", [H, 1], fp32, kind="ExternalInput")
    d_ow = nc.dram_tensor("ow", [H + 1, 2], fp32, kind="ExternalInput")
    d_blk0 = nc.dram_tensor("blk0", [AUG, BL], fp32, kind="ExternalInput")
    d_vT = nc.dram_tensor("vT", [3, T * BL], fp32, kind="ExternalInput")
    d_ones = nc.dram_tensor("ones_tail", [1, BL], fp32, kind="ExternalInput")
    d_out = nc.dram_tensor("outT", [2, T * BL], fp32, kind="ExternalOutput")

    RW = (TC + 1) * BL    # region width
    XW = TC * BL          # xn buffer width

    with TileContext(nc) as tc:
        with (
            tc.tile_pool(name="const", bufs=1) as cpool,
            tc.tile_pool(name="reg", bufs=1) as rpool,
            tc.tile_pool(name="xn", bufs=1) as xpool,
            tc.tile_pool(name="gates", bufs=4) as gpool,
            tc.tile_pool(name="psg", bufs=4, space="PSUM") as pgpool,
            tc.tile_pool(name="psx", bufs=2, space="PSUM") as pxpool,
            tc.tile_pool(name="pso", bufs=2, space="PSUM") as popool,
            tc.tile_pool(name="outs", bufs=4) as opool,
        ):
            # --- constants into SBUF ---
            wr = cpool.tile([AUG, H], fp32, tag="wr")
            wz = cpool.tile([AUG, H], fp32, tag="wz")
            wn = cpool.tile([AUG, H], fp32, tag="wn")
            xw = cpool.tile([2, H], fp32, tag="xw")
            xb = cpool.tile([H, 1], fp32, tag="xb")
            ow = cpool.tile([H + 1, 2], fp32, tag="ow")
            nc.sync.dma_start(out=wr, in_=d_wr[:, :])
            nc.sync.dma_start(out=wz, in_=d_wz[:, :])
            nc.sync.dma_start(out=wn, in_=d_wn[:, :])
            nc.sync.dma_start(out=xw, in_=d_xw[:, :])
            nc.sync.dma_start(out=xb, in_=d_xb[:, :])
            nc.sync.dma_start(out=ow, in_=d_ow[:, :])

            # --- persistent ping-pong buffers ---
            regs = [
                rpool.tile([AUG, RW], fp32, name="regA", tag="regA"),
                rpool.tile([AUG, RW], fp32, name="regB", tag="regB"),
            ]
            vxs = [
                xpool.tile([2, XW], fp32, name="vxA", tag="vxA"),
                xpool.tile([2, XW], fp32, name="vxB", tag="vxB"),
            ]
            xns = [
                xpool.tile([H, XW], fp32, name="xnA", tag="xnA"),
                xpool.tile([H, XW], fp32, name="xnB", tag="xnB"),
            ]
            # ones row of the trailing block (written once per region)
            nc.sync.dma_start(
                out=regs[0][ONES_ROW : ONES_ROW + 1, XW:RW], in_=d_ones[:, :]
            )
            nc.sync.dma_start(
                out=regs[1][ONES_ROW : ONES_ROW + 1, XW:RW], in_=d_ones[:, :]
            )
            # chunk 0 block 0: pre-assembled [h0; 1; v_0]
            nc.sync.dma_start(out=regs[0][0:AUG, 0:BL], in_=d_blk0[:, :])

            def load_chunk_v(k):
                """DMA chunk k's [ones; v] rows into the region + v copy for xn."""
                c0 = 0 if k > 0 else BL  # chunk 0 block 0 comes from blk0
                nc.sync.dma_start(
                    out=regs[k % 2][ONES_ROW:AUG, c0:XW],
                    in_=d_vT[:, k * XW + c0 : (k + 1) * XW],
                )
                nc.sync.dma_start(
                    out=vxs[k % 2][:, :], in_=d_vT[1:3, k * XW : (k + 1) * XW]
                )

            def compute_xn(k):
                """xn = W_in @ v + b_in for chunk k (K=2 matmuls), into SBUF."""
                vx = vxs[k % 2]
                xnb = xns[k % 2]
                for j in range(0, XW, 512):
                    w = min(512, XW - j)
                    ps = pxpool.tile([H, 512], fp32, tag="psx")
                    nc.tensor.matmul(
                        ps[:, :w], xw, vx[:, j : j + w], start=True, stop=True
                    )
                    # PSUM -> SBUF with the b_in bias folded in
                    nc.scalar.add(out=xnb[:, j : j + w], in_=ps[:, :w], add=xb[:, 0:1])

            load_chunk_v(0)
            compute_xn(0)

            for k in range(NCHUNK):
                reg = regs[k % 2]
                xnb = xns[k % 2]

                # prefetch next chunk's v + xn while this chunk scans
                if k + 1 < NCHUNK:
                    load_chunk_v(k + 1)
                    compute_xn(k + 1)

                for t in range(TC):
                    c0 = t * BL
                    c1 = c0 + BL
                    rhs = reg[0:AUG, c0:c1]
                    pg = pgpool.tile([H, 3 * BL], fp32, name="pg", tag="pg")
                    nc.tensor.matmul(pg[:, 0:BL], wr, rhs, start=True, stop=True)
                    nc.tensor.matmul(
                        pg[:, BL : 2 * BL], wz, rhs, start=True, stop=True
                    )
                    nc.tensor.matmul(
                        pg[:, 2 * BL : 3 * BL], wn, rhs, start=True, stop=True
                    )
                    sig = gpool.tile([H, 2 * BL], fp32, name="sig", tag="sig")
                    nc.scalar.activation(
                        sig, pg[:, 0 : 2 * BL], mybir.ActivationFunctionType.Sigmoid
                    )
                    m1 = gpool.tile([H, BL], fp32, name="m1", tag="m1")
                    nc.vector.tensor_mul(m1, sig[:, 0:BL], pg[:, 2 * BL : 3 * BL])
                    npre = gpool.tile([H, BL], fp32, name="npre", tag="npre")
                    nc.vector.tensor_add(npre, m1, xnb[:, c0:c1])
                    # pre-tanh (fills the tanh wait): d = h - zbar*h
                    q1 = gpool.tile([H, BL], fp32, name="q1", tag="q1")
                    nc.vector.tensor_mul(q1, sig[:, BL : 2 * BL], reg[0:H, c0:c1])
                    d = gpool.tile([H, BL], fp32, name="d", tag="d")
                    nc.vector.scalar_tensor_tensor(
                        d, q1, -1.0, reg[0:H, c0:c1],
                        mybir.AluOpType.mult, mybir.AluOpType.add,
                    )
                    nt = gpool.tile([H, BL], fp32, name="nt", tag="nt")
                    nc.scalar.activation(
                        nt, npre, mybir.ActivationFunctionType.Tanh
                    )
                    # post-tanh: h' = zbar*n + d -> next rhs block
                    q2 = gpool.tile([H, BL], fp32, name="q2", tag="q2")
                    nc.vector.tensor_mul(q2, sig[:, BL : 2 * BL], nt)
                    nc.vector.tensor_add(reg[0:H, c1 : c1 + BL], q2, d)

                # carry h into next chunk's block 0
                if k + 1 < NCHUNK:
                    nc.vector.tensor_copy(
                        regs[(k + 1) % 2][0:H, 0:BL], reg[0:H, TC * BL : RW]
                    )

                # output projection for this chunk: blocks 1..TC hold
                # h_t for global steps k*TC .. k*TC+TC-1
                for j in range(0, XW, 512):
                    w = min(512, XW - j)
                    po = popool.tile([2, 512], fp32, tag="po")
                    nc.tensor.matmul(
                        po[:, :w],
                        ow,
                        reg[0 : H + 1, BL + j : BL + j + w],
                        start=True,
                        stop=True,
                    )
                    ob = opool.tile([2, 512], fp32, tag="ob")
                    nc.vector.tensor_copy(ob[:, :w], po[:, :w])
                    nc.sync.dma_start(
                        out=d_out[:, k * XW + j : k * XW + j + w], in_=ob[:, :w]
                    )

    return nc


def _prep_inputs(x_i, v, w_ih, w_hh, b_ih, b_hh, w_out, b_out):
    """Host-side prep: h0 least-squares init + per-core shards."""
    f = np.float32
    x_i, v = np.asarray(x_i, f), np.asarray(v, f)
    w_ih, w_hh = np.asarray(w_ih, f), np.asarray(w_hh, f)
    b_ih, b_hh = np.asarray(b_ih, f), np.asarray(b_hh, f)
    w_out, b_out = np.asarray(w_out, f), np.asarray(b_out, f)

    A = w_out @ w_out.T
    bb = np.linalg.solve(A.astype(np.float64), (x_i - b_out).T.astype(np.float64))
    h0 = (bb.T @ w_out.astype(np.float64)).astype(f)  # [B, H]

    def aug(g0, with_ih):
        whh = w_hh[g0 : g0 + H]                      # [H, H]
        if with_ih:
            b = b_ih[g0 : g0 + H] + b_hh[g0 : g0 + H]
            wih = w_ih[g0 : g0 + H]                  # [H, 2]
        else:
            b = b_hh[g0 : g0 + H]
            wih = np.zeros((H, 2), f)
        out = np.empty((AUG, H), f)
        out[0:H] = whh.T
        out[ONES_ROW] = b
        out[V_ROW : V_ROW + 2] = wih.T
        return np.ascontiguousarray(out)

    wr = aug(0, True)
    wz = np.ascontiguousarray(-aug(H, True))
    wn = aug(2 * H, False)
    xw = np.ascontiguousarray(w_ih[2 * H : 3 * H].T)          # [2, H]
    xb = np.ascontiguousarray(b_ih[2 * H : 3 * H].reshape(H, 1))
    ow = np.empty((H + 1, 2), f)
    ow[0:H] = w_out.T
    ow[H] = b_out
    ow = np.ascontiguousarray(ow)
    ones_tail = np.ones((1, BL), f)

    in_maps = []
    for c in range(NCORES):
        rows = slice(c * BL, (c + 1) * BL)
        vT = np.empty((3, T * BL), f)
        vT[0] = 1.0
        vT[1:3] = v[rows].transpose(2, 1, 0).reshape(2, T * BL)
        vT = np.ascontiguousarray(vT)
        blk0 = np.empty((AUG, BL), f)
        blk0[0:H] = h0[rows].T
        blk0[ONES_ROW] = 1.0
        blk0[V_ROW : V_ROW + 2] = vT[1:3, 0:BL]
        blk0 = np.ascontiguousarray(blk0)
        in_maps.append(
            {
                "wr": wr, "wz": wz, "wn": wn, "xw": xw, "xb": xb, "ow": ow,
                "blk0": blk0, "vT": vT, "ones_tail": ones_tail,
            }
        )
    return in_maps


def kernel(x_i, v, w_ih, w_hh, b_ih, b_hh, w_out, b_out, trace=False, tmpdir=None):
    global _compiled
    from concourse.bass_utils import run_bass_kernel_spmd

    in_maps = _prep_inputs(x_i, v, w_ih, w_hh, b_ih, b_hh, w_out, b_out)
    if _compiled is None:
        _compiled = _build_kernel()
        _compiled.finalize()
    kw = {}
    if trace:
        kw = dict(trace=True, tmpdir=tmpdir)
    res = run_bass_kernel_spmd(
        _compiled, in_maps, core_ids=list(range(NCORES)), **kw
    )
    out = np.empty((B, T, 2), np.float32)
    for c in range(NCORES):
        outT = res.results[c]["outT"]                              # [2, T*BL]
        out[c * BL : (c + 1) * BL] = outT.reshape(2, T, BL).transpose(2, 1, 0)
    kernel.last_results = res
    return out


# revision 13
# speedup vs baseline: 1.0826x; 1.0027x over previous
"""GRU (B=256, T=2048, H=100) Trainium2 kernel.

Data-parallel over batch: 8 NeuronCores x 32 batch rows each. The
sequential scan over T=2048 steps runs device-local on each core.

Layout: hidden state kept transposed [H=100 partitions, B=32 free].
Per step, three matmuls with augmented stationary weights
lhsT_g = [W_hh_g.T ; b_g ; W_ih_g.T]  (K=103, M=100) against
rhs = [h_{t-1} ; 1 ; v_t] [103, 32] produce r_pre | z_pre | hn in one
PSUM tile; sigmoid/tanh on ScalarE, gate arithmetic on VectorE. Each
step's h_t is written into the next step's rhs block of a persistent
SBUF region, which doubles as the hs buffer for the output projection
(out = w_out @ h + b_out, bias folded via the region's ones row).
"""

import sys

sys.path.insert(0, "/opt/trn_rl_repo")

import numpy as np

B, T, H = 256, 2048, 100
NCORES = 8
BL = B // NCORES          # 32 batch rows per core
TC = 128                  # steps per chunk
NCHUNK = T // TC
AUG = H + 3               # h rows + ones row + 2 v rows
ONES_ROW = H              # partition index of the ones row
V_ROW = H + 1             # partition index of first v row

_compiled = None


def _build_kernel():
    import concourse.mybir as mybir
    from concourse import bacc
    from concourse.tile import TileContext

    fp32 = mybir.dt.float32
    nc = bacc.Bacc(None, target_bir_lowering=False)

    # DRAM I/O (per-core tensors; same names across cores for SPMD)
    d_wr = nc.dram_tensor("wr", [AUG, H], fp32, kind="ExternalInput")
    d_wz = nc.dram_tensor("wz", [AUG, H], fp32, kind="ExternalInput")
    d_wn = nc.dram_tensor("wn", [AUG, H], fp32, kind="ExternalInput")
    d_xw = nc.dram_tensor("xw", [2, H], fp32, kind="ExternalInput")
    d_xb = nc.dram_tensor("xb", [H, 1], fp32, kind="ExternalInput")
    d_ow = nc.dram_tensor("ow", [H + 1, 2], fp32, kind="ExternalInput")
    d_blk0 = nc.dram_tensor("blk0", [AUG, BL], fp32, kind="ExternalInput")
    d_vT = nc.dram_tensor("vT", [3, T * BL], fp32, kind="ExternalInput")
    d_ones = nc.dram_tensor("ones_tail", [1, BL], fp32, kind="ExternalInput")
    d_out = nc.dram_tensor("outT", [2, T * BL], fp32, kind="ExternalOutput")

    RW = (TC + 1) * BL    # region width
    XW = TC * BL          # xn buffer width

    with TileContext(nc) as tc:
        with (
            tc.tile_pool(name="const", bufs=1) as cpool,
            tc.tile_pool(name="reg", bufs=1) as rpool,
            tc.tile_pool(name="xn", bufs=1) as xpool,
            tc.tile_pool(name="gates", bufs=4) as gpool,
            tc.tile_pool(name="psg", bufs=4, space="PSUM") as pgpool,
            tc.tile_pool(name="psx", bufs=2, space="PSUM") as pxpool,
            tc.tile_pool(name="pso", bufs=2, space="PSUM") as popool,
            tc.tile_pool(name="outs", bufs=4) as opool,
        ):
            # --- constants into SBUF ---
            wr = cpool.tile([AUG, H], fp32, tag="wr")
            wz = cpool.tile([AUG, H], fp32, tag="wz")
            wn = cpool.tile([AUG, H], fp32, tag="wn")
            xw = cpool.tile([2, H], fp32, tag="xw")
            xb = cpool.tile([H, 1], fp32, tag="xb")
            ow = cpool.tile([H + 1, 2], fp32, tag="ow")
            nc.sync.dma_start(out=wr, in_=d_wr[:, :])
            nc.sync.dma_start(out=wz, in_=d_wz[:, :])
            nc.sync.dma_start(out=wn, in_=d_wn[:, :])
            nc.sync.dma_start(out=xw, in_=d_xw[:, :])
            nc.sync.dma_start(out=xb, in_=d_xb[:, :])
            nc.sync.dma_start(out=ow, in_=d_ow[:, :])

            # --- persistent ping-pong buffers ---
            regs = [
                rpool.tile([AUG, RW], fp32, name="regA", tag="regA"),
                rpool.tile([AUG, RW], fp32, name="regB", tag="regB"),
            ]
            vxs = [
                xpool.tile([2, XW], fp32, name="vxA", tag="vxA"),
                xpool.tile([2, XW], fp32, name="vxB", tag="vxB"),
            ]
            xns = [
                xpool.tile([H, XW], fp32, name="xnA", tag="xnA"),
                xpool.tile([H, XW], fp32, name="xnB", tag="xnB"),
            ]
            # ones row of the trailing block (written once per region)
            nc.sync.dma_start(
                out=regs[0][ONES_ROW : ONES_ROW + 1, XW:RW], in_=d_ones[:, :]
            )
            nc.sync.dma_start(
                out=regs[1][ONES_ROW : ONES_ROW + 1, XW:RW], in_=d_ones[:, :]
            )
            # chunk 0 block 0: pre-assembled [h0; 1; v_0]
            nc.sync.dma_start(out=regs[0][0:AUG, 0:BL], in_=d_blk0[:, :])

            def load_chunk_v(k):
                """DMA chunk k's [ones; v] rows into the region + v copy for xn."""
                c0 = 0 if k > 0 else BL  # chunk 0 block 0 comes from blk0
                nc.sync.dma_start(
                    out=regs[k % 2][ONES_ROW:AUG, c0:XW],
                    in_=d_vT[:, k * XW + c0 : (k + 1) * XW],
                )
                nc.sync.dma_start(
                    out=vxs[k % 2][:, :], in_=d_vT[1:3, k * XW : (k + 1) * XW]
                )

            def compute_xn(k):
                """xn = W_in @ v + b_in for chunk k (K=2 matmuls), into SBUF."""
                vx = vxs[k % 2]
                xnb = xns[k % 2]
                for j in range(0, XW, 512):
                    w = min(512, XW - j)
                    ps = pxpool.tile([H, 512], fp32, tag="psx")
                    nc.tensor.matmul(
                        ps[:, :w], xw, vx[:, j : j + w], start=True, stop=True
                    )
                    # PSUM -> SBUF with the b_in bias folded in
                    nc.scalar.add(out=xnb[:, j : j + w], in_=ps[:, :w], add=xb[:, 0:1])

            load_chunk_v(0)
            compute_xn(0)

            for k in range(NCHUNK):
                reg = regs[k % 2]
                xnb = xns[k % 2]

                # prefetch next chunk's v + xn while this chunk scans
                if k + 1 < NCHUNK:
                    load_chunk_v(k + 1)
                    compute_xn(k + 1)

                for t in range(TC):
                    c0 = t * BL
                    c1 = c0 + BL
                    rhs = reg[0:AUG, c0:c1]
                    pg = pgpool.tile([H, 3 * BL], fp32, name="pg", tag="pg")
                    nc.tensor.matmul(pg[:, 0:BL], wr, rhs, start=True, stop=True)
                    nc.tensor.matmul(
                        pg[:, BL : 2 * BL], wz, rhs, start=True, stop=True
                    )
                    nc.tensor.matmul(
                        pg[:, 2 * BL : 3 * BL], wn, rhs, start=True, stop=True
                    )
                    sig = gpool.tile([H, 2 * BL], fp32, name="sig", tag="sig")
                    nc.scalar.activation(
                        sig, pg[:, 0 : 2 * BL], mybir.ActivationFunctionType.Sigmoid
                    )
                    m1 = gpool.tile([H, BL], fp32, name="m1", tag="m1")
                    nc.vector.tensor_mul(m1, sig[:, 0:BL], pg[:, 2 * BL : 3 * BL])
                    npre = gpool.tile([H, BL], fp32, name="npre", tag="npre")
                    nc.vector.tensor_add(npre, m1, xnb[:, c0:c1])
                    # pre-tanh (fills the tanh wait): d = h - zbar*h
                    q1 = gpool.tile([H, BL], fp32, name="q1", tag="q1")
                    nc.vector.tensor_mul(q1, sig[:, BL : 2 * BL], reg[0:H, c0:c1])
                    d = gpool.tile([H, BL], fp32, name="d", tag="d")
                    nc.vector.scalar_tensor_tensor(
                        d, q1, -1.0, reg[0:H, c0:c1],
                        mybir.AluOpType.mult, mybir.AluOpType.add,
                    )
                    nt = gpool.tile([H, BL], fp32, name="nt", tag="nt")
                    nc.scalar.activation(
                        nt, npre, mybir.ActivationFunctionType.Tanh
                    )
                    # post-tanh: h' = zbar*n + d -> next rhs block
                    q2 = gpool.tile([H, BL], fp32, name="q2", tag="q2")
                    nc.vector.tensor_mul(q2, sig[:, BL : 2 * BL], nt)
                    nc.vector.tensor_add(reg[0:H, c1 : c1 + BL], q2, d)

                # carry h into next chunk's block 0
                if k + 1 < NCHUNK:
                    nc.vector.tensor_copy(
                        regs[(k + 1) % 2][0:H, 0:BL], reg[0:H, TC * BL : RW]
                    )

                # output projection for this chunk: blocks 1..TC hold
                # h_t for global steps k*TC .. k*TC+TC-1
                for j in range(0, XW, 512):
                    w = min(512, XW - j)
                    po = popool.tile([2, 512], fp32, tag="po")
                    nc.tensor.matmul(
                        po[:, :w],
                        ow,
                        reg[0 : H + 1, BL + j : BL + j + w],
                        start=True,
                        stop=True,
                    )
                    ob = opool.tile([2, 512], fp32, tag="ob")
                    nc.scalar.copy(out=ob[:, :w], in_=po[:, :w])
                    nc.sync.dma_start(
                        out=d_out[:, k * XW + j : k * XW + j + w], in_=ob[:, :w]
                    )

    return nc


def _prep_inputs(x_i, v, w_ih, w_hh, b_ih, b_hh, w_out, b_out):
    """Host-side prep: h0 least-squares init + per-core shards."""
    f = np.float32
    x_i, v = np.asarray(x_i, f), np.asarray(v, f)
    w_ih, w_hh = np.asarray(w_ih, f), np.asarray(w_hh, f)
    b_ih, b_hh = np.asarray(b_ih, f), np.asarray(b_hh, f)
    w_out, b_out = np.asarray(w_out, f), np.asarray(b_out, f)

    A = w_out @ w_out.T
    bb = np.linalg.solve(A.astype(np.float64), (x_i - b_out).T.astype(np.float64))
    h0 = (bb.T @ w_out.astype(np.float64)).astype(f)  # [B, H]

    def aug(g0, with_ih):
        whh = w_hh[g0 : g0 + H]                      # [H, H]
        if with_ih:
            b = b_ih[g0 : g0 + H] + b_hh[g0 : g0 + H]
            wih = w_ih[g0 : g0 + H]                  # [H, 2]
        else:
            b = b_hh[g0 : g0 + H]
            wih = np.zeros((H, 2), f)
        out = np.empty((AUG, H), f)
        out[0:H] = whh.T
        out[ONES_ROW] = b
        out[V_ROW : V_ROW + 2] = wih.T
        return np.ascontiguousarray(out)

    wr = aug(0, True)
    wz = np.ascontiguousarray(-aug(H, True))
    wn = aug(2 * H, False)
    xw = np.ascontiguousarray(w_ih[2 * H : 3 * H].T)          # [2, H]
    xb = np.ascontiguousarray(b_ih[2 * H : 3 * H].reshape(H, 1))
    ow = np.empty((H + 1, 2), f)
    ow[0:H] = w_out.T
    ow[H] = b_out
    ow = np.ascontiguousarray(ow)
    ones_tail = np.ones((1, BL), f)

    in_maps = []
    for c in range(NCORES):
        rows = slice(c * BL, (c + 1) * BL)
        vT = np.empty((3, T * BL), f)
        vT[0] = 1.0
        vT[1:3] = v[rows].transpose(2, 1, 0).reshape(2, T * BL)
        vT = np.ascontiguousarray(vT)
        blk0 = np.empty((AUG, BL), f)
        blk0[0:H] = h0[rows].T
        blk0[ONES_ROW] = 1.0
        blk0[V_ROW : V_ROW + 2] = vT[1:3, 0:BL]
        blk0 = np.ascontiguousarray(blk0)
        in_maps.append(
            {
                "wr": wr, "wz": wz, "wn": wn, "xw": xw, "xb": xb, "ow": ow,
                "blk0": blk0, "vT": vT, "ones_tail": ones_tail,
            }
        )
    return in_maps


def kernel(x_i, v, w_ih, w_hh, b_ih, b_hh, w_out, b_out, trace=False, tmpdir=None):
    global _compiled
    from concourse.bass_utils import run_bass_kernel_spmd

    in_maps = _prep_inputs(x_i, v, w_ih, w_hh, b_ih, b_hh, w_out, b_out)
    if _compiled is None:
        _compiled = _build_kernel()
        _compiled.finalize()
    kw = {}
    if trace:
        kw = dict(trace=True, tmpdir=tmpdir)
    res = run_bass_kernel_spmd(
        _compiled, in_maps, core_ids=list(range(NCORES)), **kw
    )
    out = np.empty((B, T, 2), np.float32)
    for c in range(NCORES):
        outT = res.results[c]["outT"]                              # [2, T*BL]
        out[c * BL : (c + 1) * BL] = outT.reshape(2, T, BL).transpose(2, 1, 0)
    kernel.last_results = res
    return out


# revision 14
# speedup vs baseline: 1.1103x; 1.0256x over previous
"""GRU (B=256, T=2048, H=100) Trainium2 kernel.

Data-parallel over batch: 8 NeuronCores x 32 batch rows each. The
sequential scan over T=2048 steps runs device-local on each core.

Layout: hidden state kept transposed [H=100 partitions, B=32 free].
Per step, three matmuls with augmented stationary weights
lhsT_g = [W_hh_g.T ; b_g ; W_ih_g.T]  (K=103, M=100) against
rhs = [h_{t-1} ; 1 ; v_t] [103, 32] produce r_pre | z_pre | hn in one
PSUM tile; sigmoid/tanh on ScalarE, gate arithmetic on VectorE. Each
step's h_t is written into the next step's rhs block of a persistent
SBUF region, which doubles as the hs buffer for the output projection
(out = w_out @ h + b_out, bias folded via the region's ones row).
"""

import sys

sys.path.insert(0, "/opt/trn_rl_repo")

import numpy as np

B, T, H = 256, 2048, 100
NCORES = 8
BL = B // NCORES          # 32 batch rows per core
TC = 128                  # steps per chunk
NCHUNK = T // TC
AUG = H + 3               # h rows + ones row + 2 v rows
ONES_ROW = H              # partition index of the ones row
V_ROW = H + 1             # partition index of first v row

_compiled = None


def _build_kernel():
    import concourse.mybir as mybir
    from concourse import bacc
    from concourse.tile import TileContext

    fp32 = mybir.dt.float32
    nc = bacc.Bacc(None, target_bir_lowering=False)

    # DRAM I/O (per-core tensors; same names across cores for SPMD)
    d_wr = nc.dram_tensor("wr", [AUG, H], fp32, kind="ExternalInput")
    d_wz = nc.dram_tensor("wz", [AUG, H], fp32, kind="ExternalInput")
    d_wn = nc.dram_tensor("wn", [AUG, H], fp32, kind="ExternalInput")
    d_xw = nc.dram_tensor("xw", [2, H], fp32, kind="ExternalInput")
    d_xb = nc.dram_tensor("xb", [H, 1], fp32, kind="ExternalInput")
    d_ow = nc.dram_tensor("ow", [H + 1, 2], fp32, kind="ExternalInput")
    d_blk0 = nc.dram_tensor("blk0", [AUG, BL], fp32, kind="ExternalInput")
    d_vT = nc.dram_tensor("vT", [3, T * BL], fp32, kind="ExternalInput")
    d_ones = nc.dram_tensor("ones_tail", [1, BL], fp32, kind="ExternalInput")
    d_out = nc.dram_tensor("outT", [2, T * BL], fp32, kind="ExternalOutput")

    RW = (TC + 1) * BL    # region width
    XW = TC * BL          # xn buffer width

    with TileContext(nc) as tc:
        with (
            tc.tile_pool(name="const", bufs=1) as cpool,
            tc.tile_pool(name="reg", bufs=1) as rpool,
            tc.tile_pool(name="xn", bufs=1) as xpool,
            tc.tile_pool(name="gates", bufs=4) as gpool,
            tc.tile_pool(name="psg", bufs=3, space="PSUM") as pgpool,
            tc.tile_pool(name="psh", bufs=3, space="PSUM") as phpool,
            tc.tile_pool(name="psx", bufs=1, space="PSUM") as pxpool,
            tc.tile_pool(name="pso", bufs=1, space="PSUM") as popool,
            tc.tile_pool(name="outs", bufs=4) as opool,
        ):
            # --- constants into SBUF ---
            wr = cpool.tile([AUG, H], fp32, tag="wr")
            wz = cpool.tile([AUG, H], fp32, tag="wz")
            wn = cpool.tile([AUG, H], fp32, tag="wn")
            xw = cpool.tile([2, H], fp32, tag="xw")
            xb = cpool.tile([H, 1], fp32, tag="xb")
            ow = cpool.tile([H + 1, 2], fp32, tag="ow")
            nc.sync.dma_start(out=wr, in_=d_wr[:, :])
            nc.sync.dma_start(out=wz, in_=d_wz[:, :])
            nc.sync.dma_start(out=wn, in_=d_wn[:, :])
            nc.sync.dma_start(out=xw, in_=d_xw[:, :])
            nc.sync.dma_start(out=xb, in_=d_xb[:, :])
            nc.sync.dma_start(out=ow, in_=d_ow[:, :])

            # --- persistent ping-pong buffers ---
            regs = [
                rpool.tile([AUG, RW], fp32, name="regA", tag="regA"),
                rpool.tile([AUG, RW], fp32, name="regB", tag="regB"),
            ]
            vxs = [
                xpool.tile([2, XW], fp32, name="vxA", tag="vxA"),
                xpool.tile([2, XW], fp32, name="vxB", tag="vxB"),
            ]
            xns = [
                xpool.tile([H, XW], fp32, name="xnA", tag="xnA"),
                xpool.tile([H, XW], fp32, name="xnB", tag="xnB"),
            ]
            # ones row of the trailing block (written once per region)
            nc.sync.dma_start(
                out=regs[0][ONES_ROW : ONES_ROW + 1, XW:RW], in_=d_ones[:, :]
            )
            nc.sync.dma_start(
                out=regs[1][ONES_ROW : ONES_ROW + 1, XW:RW], in_=d_ones[:, :]
            )
            # chunk 0 block 0: pre-assembled [h0; 1; v_0]
            nc.sync.dma_start(out=regs[0][0:AUG, 0:BL], in_=d_blk0[:, :])

            def load_chunk_v(k):
                """DMA chunk k's [ones; v] rows into the region + v copy for xn."""
                c0 = 0 if k > 0 else BL  # chunk 0 block 0 comes from blk0
                nc.sync.dma_start(
                    out=regs[k % 2][ONES_ROW:AUG, c0:XW],
                    in_=d_vT[:, k * XW + c0 : (k + 1) * XW],
                )
                nc.sync.dma_start(
                    out=vxs[k % 2][:, :], in_=d_vT[1:3, k * XW : (k + 1) * XW]
                )

            def compute_xn(k):
                """xn = W_in @ v + b_in for chunk k (K=2 matmuls), into SBUF."""
                vx = vxs[k % 2]
                xnb = xns[k % 2]
                for j in range(0, XW, 512):
                    w = min(512, XW - j)
                    ps = pxpool.tile([H, 512], fp32, tag="psx")
                    nc.tensor.matmul(
                        ps[:, :w], xw, vx[:, j : j + w], start=True, stop=True
                    )
                    # PSUM -> SBUF with the b_in bias folded in
                    nc.scalar.add(out=xnb[:, j : j + w], in_=ps[:, :w], add=xb[:, 0:1])

            load_chunk_v(0)
            compute_xn(0)

            for k in range(NCHUNK):
                reg = regs[k % 2]
                xnb = xns[k % 2]

                # prefetch next chunk's v + xn while this chunk scans
                if k + 1 < NCHUNK:
                    load_chunk_v(k + 1)
                    compute_xn(k + 1)

                for t in range(TC):
                    c0 = t * BL
                    c1 = c0 + BL
                    rhs = reg[0:AUG, c0:c1]
                    pg = pgpool.tile([H, 2 * BL], fp32, name="pg", tag="pg")
                    ph = phpool.tile([H, BL], fp32, name="ph", tag="ph")
                    nc.tensor.matmul(pg[:, 0:BL], wr, rhs, start=True, stop=True)
                    nc.tensor.matmul(
                        pg[:, BL : 2 * BL], wz, rhs, start=True, stop=True
                    )
                    # hn in its own PSUM tile so sigma only waits on r|z
                    nc.tensor.matmul(ph, wn, rhs, start=True, stop=True)
                    sig = gpool.tile([H, 2 * BL], fp32, name="sig", tag="sig")
                    nc.scalar.activation(
                        sig, pg[:, 0 : 2 * BL], mybir.ActivationFunctionType.Sigmoid
                    )
                    m1 = gpool.tile([H, BL], fp32, name="m1", tag="m1")
                    nc.vector.tensor_mul(m1, sig[:, 0:BL], ph)
                    npre = gpool.tile([H, BL], fp32, name="npre", tag="npre")
                    nc.vector.tensor_add(npre, m1, xnb[:, c0:c1])
                    # pre-tanh (fills the tanh wait): d = h - zbar*h
                    q1 = gpool.tile([H, BL], fp32, name="q1", tag="q1")
                    nc.vector.tensor_mul(q1, sig[:, BL : 2 * BL], reg[0:H, c0:c1])
                    d = gpool.tile([H, BL], fp32, name="d", tag="d")
                    nc.vector.scalar_tensor_tensor(
                        d, q1, -1.0, reg[0:H, c0:c1],
                        mybir.AluOpType.mult, mybir.AluOpType.add,
                    )
                    nt = gpool.tile([H, BL], fp32, name="nt", tag="nt")
                    nc.scalar.activation(
                        nt, npre, mybir.ActivationFunctionType.Tanh
                    )
                    # post-tanh: h' = zbar*n + d -> next rhs block
                    q2 = gpool.tile([H, BL], fp32, name="q2", tag="q2")
                    nc.vector.tensor_mul(q2, sig[:, BL : 2 * BL], nt)
                    nc.vector.tensor_add(reg[0:H, c1 : c1 + BL], q2, d)

                # carry h into next chunk's block 0
                if k + 1 < NCHUNK:
                    nc.vector.tensor_copy(
                        regs[(k + 1) % 2][0:H, 0:BL], reg[0:H, TC * BL : RW]
                    )

                # output projection for this chunk: blocks 1..TC hold
                # h_t for global steps k*TC .. k*TC+TC-1
                for j in range(0, XW, 512):
                    w = min(512, XW - j)
                    po = popool.tile([2, 512], fp32, tag="po")
                    nc.tensor.matmul(
                        po[:, :w],
                        ow,
                        reg[0 : H + 1, BL + j : BL + j + w],
                        start=True,
                        stop=True,
                    )
                    ob = opool.tile([2, 512], fp32, tag="ob")
                    nc.scalar.copy(out=ob[:, :w], in_=po[:, :w])
                    nc.sync.dma_start(
                        out=d_out[:, k * XW + j : k * XW + j + w], in_=ob[:, :w]
                    )

    return nc


def _prep_inputs(x_i, v, w_ih, w_hh, b_ih, b_hh, w_out, b_out):
    """Host-side prep: h0 least-squares init + per-core shards."""
    f = np.float32
    x_i, v = np.asarray(x_i, f), np.asarray(v, f)
    w_ih, w_hh = np.asarray(w_ih, f), np.asarray(w_hh, f)
    b_ih, b_hh = np.asarray(b_ih, f), np.asarray(b_hh, f)
    w_out, b_out = np.asarray(w_out, f), np.asarray(b_out, f)

    A = w_out @ w_out.T
    bb = np.linalg.solve(A.astype(np.float64), (x_i - b_out).T.astype(np.float64))
    h0 = (bb.T @ w_out.astype(np.float64)).astype(f)  # [B, H]

    def aug(g0, with_ih):
        whh = w_hh[g0 : g0 + H]                      # [H, H]
        if with_ih:
            b = b_ih[g0 : g0 + H] + b_hh[g0 : g0 + H]
            wih = w_ih[g0 : g0 + H]                  # [H, 2]
        else:
            b = b_hh[g0 : g0 + H]
            wih = np.zeros((H, 2), f)
        out = np.empty((AUG, H), f)
        out[0:H] = whh.T
        out[ONES_ROW] = b
        out[V_ROW : V_ROW + 2] = wih.T
        return np.ascontiguousarray(out)

    wr = aug(0, True)
    wz = np.ascontiguousarray(-aug(H, True))
    wn = aug(2 * H, False)
    xw = np.ascontiguousarray(w_ih[2 * H : 3 * H].T)          # [2, H]
    xb = np.ascontiguousarray(b_ih[2 * H : 3 * H].reshape(H, 1))
    ow = np.empty((H + 1, 2), f)
    ow[0:H] = w_out.T
    ow[H] = b_out
    ow = np.ascontiguousarray(ow)
    ones_tail = np.ones((1, BL), f)

    in_maps = []
    for c in range(NCORES):
        rows = slice(c * BL, (c + 1) * BL)
        vT = np.empty((3, T * BL), f)
        vT[0] = 1.0
        vT[1:3] = v[rows].transpose(2, 1, 0).reshape(2, T * BL)
        vT = np.ascontiguousarray(vT)
        blk0 = np.empty((AUG, BL), f)
        blk0[0:H] = h0[rows].T
        blk0[ONES_ROW] = 1.0
        blk0[V_ROW : V_ROW + 2] = vT[1:3, 0:BL]
        blk0 = np.ascontiguousarray(blk0)
        in_maps.append(
            {
                "wr": wr, "wz": wz, "wn": wn, "xw": xw, "xb": xb, "ow": ow,
                "blk0": blk0, "vT": vT, "ones_tail": ones_tail,
            }
        )
    return in_maps


def kernel(x_i, v, w_ih, w_hh, b_ih, b_hh, w_out, b_out, trace=False, tmpdir=None):
    global _compiled
    from concourse.bass_utils import run_bass_kernel_spmd

    in_maps = _prep_inputs(x_i, v, w_ih, w_hh, b_ih, b_hh, w_out, b_out)
    if _compiled is None:
        _compiled = _build_kernel()
        _compiled.finalize()
    kw = {}
    if trace:
        kw = dict(trace=True, tmpdir=tmpdir)
    res = run_bass_kernel_spmd(
        _compiled, in_maps, core_ids=list(range(NCORES)), **kw
    )
    out = np.empty((B, T, 2), np.float32)
    for c in range(NCORES):
        outT = res.results[c]["outT"]                              # [2, T*BL]
        out[c * BL : (c + 1) * BL] = outT.reshape(2, T, BL).transpose(2, 1, 0)
    kernel.last_results = res
    return out
